# revision 1
# baseline (speedup 1.0000x reference)
"""GPT forward pass on 8 TRN2 NeuronCores.

Sharding: core c -> batch b = c // 2, sequence half = c % 2 (contiguous
512-token halves).  The residual stream stays core-local in a D-major
layout (h^T: [D=1024 partition-chunks, 512 own tokens]).  Once per layer,
an AllGather over core pairs exchanges the post-LN activations z^T (bf16)
so each core computes K/V over the full 1024-token sequence; Q/attention
rows/proj/MLP only cover the core's own 512 tokens.

Attention uses the S^T scheme: S^T = K_h @ Q_h^T per 128-key block so the
softmax denominator comes from a ones-column appended to V (row 64 of the
PV accumulator), and causal masking is a data-driven full-mask add (the
per-core mask input encodes hidden/diagonal/visible blocks), keeping the
program SPMD-identical across cores.  No PE transposes anywhere.

Weights are pre-cast to bf16 on the host; matmuls accumulate in f32 PSUM;
the residual stream stays f32.  Biases and LN affine params are zeros/ones
in this model and are skipped.
"""

import sys

sys.path.insert(0, "/opt/trn_rl_repo")

import numpy as np
import ml_dtypes

import concourse.bass as bass
import concourse.bacc as bacc
import concourse.mybir as mybir
from concourse import tile
from concourse.bass_utils import run_bass_kernel_spmd

B, T, E, D, NH, DH, NL, FF, AD = 4, 1024, 512, 1024, 16, 64, 8, 4096, 8
TH = T // 2          # tokens per core
NC = 8
DCH = D // 128       # 8 partition chunks of the embedding dim
TCH = TH // 128      # 4 token tiles per half
EPS = 1e-5
BF = mybir.dt.bfloat16
F32 = mybir.dt.float32
AluOp = mybir.AluOpType
Act = mybir.ActivationFunctionType

_cache = {}


def _build_program():
    nc = bacc.Bacc("TRN2", target_bir_lowering=False, debug=False, num_devices=NC)

    # --- DRAM parameters (identical graph on all cores; data differs) ---
    p_lcdT = nc.declare_dram_parameter("lcdT", [E, TH], BF, isOutput=False)
    p_actT = nc.declare_dram_parameter("actT", [AD, TH], F32, isOutput=False)
    p_posT = nc.declare_dram_parameter("posT", [D, TH], F32, isOutput=False)
    p_we = nc.declare_dram_parameter("W_embed", [E, D // 2], BF, isOutput=False)
    p_wa = nc.declare_dram_parameter("W_act", [AD, D // 2], F32, isOutput=False)
    p_wq = nc.declare_dram_parameter("Wq", [NL, D, D], BF, isOutput=False)
    p_wk = nc.declare_dram_parameter("Wk", [NL, D, D], BF, isOutput=False)
    p_wv = nc.declare_dram_parameter("Wv", [NL, D, D], BF, isOutput=False)
    p_wp = nc.declare_dram_parameter("Wp", [NL, D, D], BF, isOutput=False)
    p_w1 = nc.declare_dram_parameter("W1", [NL, D, FF], BF, isOutput=False)
    p_w2 = nc.declare_dram_parameter("W2", [NL, FF, D], BF, isOutput=False)
    p_wh = nc.declare_dram_parameter("Wh", [D, E], BF, isOutput=False)
    p_mask = nc.declare_dram_parameter("maskT", [8, 128, TH], BF, isOutput=False)
    p_out = nc.declare_dram_parameter("out", [TH, E], F32, isOutput=True)

    with tile.TileContext(nc) as tc:
        # ---------------- pools ----------------
        const = tc.alloc_tile_pool(name="const", bufs=1)
        persist = tc.alloc_tile_pool(name="persist", bufs=1)
        zpool = tc.alloc_tile_pool(name="zpool", bufs=1)
        big = tc.alloc_tile_pool(name="bigact", bufs=1)
        wpool = tc.alloc_tile_pool(name="wpool", bufs=3)
        wvpool = tc.alloc_tile_pool(name="wvpool", bufs=2)
        tmp = tc.alloc_tile_pool(name="tmp", bufs=3)
        stat = tc.alloc_tile_pool(name="stat", bufs=6)
        ptp = tc.alloc_tile_pool(name="ptp", bufs=3)
        dram = tc.alloc_tile_pool(name="dram", bufs=2, space="DRAM")
        pp_mm = tc.alloc_tile_pool(name="pp_mm", bufs=2, space="PSUM")
        pp_s = tc.alloc_tile_pool(name="pp_s", bufs=2, space="PSUM")
        pp_o = tc.alloc_tile_pool(name="pp_o", bufs=1, space="PSUM")
        pp_ln = tc.alloc_tile_pool(name="pp_ln", bufs=1, space="PSUM")

        ones_col = const.tile([128, 1], F32)
        nc.gpsimd.memset(ones_col[:], 1.0)
        ones_row = const.tile([1, 128], F32)
        nc.gpsimd.memset(ones_row[:], 1.0)
        eps_t = const.tile([1, 1], F32)
        nc.gpsimd.memset(eps_t[:], EPS)

        # residual stream h^T, f32, D-chunk d at [:, d, :]
        h = persist.tile([128, DCH, TH], F32)
        # additive causal mask in S^T layout, k-block kc at [:, kc, :]
        maskT = persist.tile([128, 8, TH], BF)
        nc.sync.dma_start(maskT[:], p_mask.ap().rearrange("k p t -> p k t"))

        QT = persist.tile([128, DCH, TH], BF)    # Q^T  rows=D, cols=own tok
        KT = persist.tile([128, DCH, T], BF)     # K^T  rows=D, cols=all tok
        VA = persist.tile([128, 8, NH * 65], BF)  # V rows=tok, 65-wide head blocks
        yT = persist.tile([128, DCH, TH], BF)    # attn out^T, rows=D

        # ---------------- helpers ----------------
        def layernorm(z_out):
            """z_out (sbuf bf16 [128, DCH, TH]) = LayerNorm(h) in D-major."""
            s_p = pp_ln.tile([1, TH], F32, tag="lnacc", bufs=1)
            for d in range(DCH):
                nc.tensor.matmul(s_p[:], ones_col[:], h[:, d, :],
                                 start=(d == 0), stop=(d == DCH - 1))
            mean = stat.tile([1, TH], F32, tag="stat")
            nc.vector.tensor_scalar_mul(mean[:], s_p[:], 1.0 / D)
            q_p = pp_ln.tile([1, TH], F32, tag="lnacc", bufs=1)
            for d in range(DCH):
                sq = tmp.tile([128, TH], F32, tag="t32")
                nc.scalar.square(sq[:], h[:, d, :])
                nc.tensor.matmul(q_p[:], ones_col[:], sq[:],
                                 start=(d == 0), stop=(d == DCH - 1))
            e2 = stat.tile([1, TH], F32, tag="stat")
            nc.vector.tensor_scalar_mul(e2[:], q_p[:], 1.0 / D)
            m2 = stat.tile([1, TH], F32, tag="stat")
            nc.scalar.square(m2[:], mean[:])
            var = stat.tile([1, TH], F32, tag="stat")
            nc.vector.tensor_sub(var[:], e2[:], m2[:])
            std = stat.tile([1, TH], F32, tag="stat")
            nc.scalar.activation(std[:], var[:], Act.Sqrt, bias=eps_t[:])
            rinv = stat.tile([1, TH], F32, tag="stat")
            nc.vector.reciprocal(rinv[:], std[:])
            nmr = stat.tile([1, TH], F32, tag="stat")
            nc.vector.tensor_mul(nmr[:], mean[:], rinv[:])
            nc.vector.tensor_scalar_mul(nmr[:], nmr[:], -1.0)
            rB = pp_ln.tile([128, TH], F32, tag="bcast", bufs=2)
            nc.tensor.matmul(rB[:], ones_row[:], rinv[:], start=True, stop=True)
            bB = pp_ln.tile([128, TH], F32, tag="bcast", bufs=2)
            nc.tensor.matmul(bB[:], ones_row[:], nmr[:], start=True, stop=True)
            for d in range(DCH):
                t = tmp.tile([128, TH], F32, tag="t32")
                nc.vector.tensor_tensor(t[:], h[:, d, :], rB[:], AluOp.mult)
                nc.vector.tensor_tensor(z_out[:, d, :], t[:], bB[:], AluOp.add)

        # ---------------- embedding ----------------
        we3 = p_we.ap().rearrange("(c p) n -> p c n", p=128)      # [128, 4, 512]
        for r in range(4):
            wet = tmp.tile([128, 4, 128], BF, tag="tbf")
            nc.sync.dma_start(wet[:], we3[:, :, r * 128:(r + 1) * 128])
            ep = pp_mm.tile([128, TH], F32, tag="mm")
            for ec in range(4):
                lt = tmp.tile([128, TH], BF, tag="tbf")
                nc.sync.dma_start(lt[:], p_lcdT.ap()[ec * 128:(ec + 1) * 128, :])
                nc.tensor.matmul(ep[:], wet[:, ec, :], lt[:],
                                 start=(ec == 0), stop=(ec == 3))
            pt = tmp.tile([128, TH], F32, tag="t32")
            nc.sync.dma_start(pt[:], p_posT.ap()[r * 128:(r + 1) * 128, :])
            nc.vector.tensor_tensor(h[:, r, :], ep[:], pt[:], AluOp.add)
        actT = tmp.tile([AD, TH], F32, tag="t32")
        nc.sync.dma_start(actT[:], p_actT.ap())
        for r in range(4):
            wat = tmp.tile([AD, 128], F32, tag="t32")
            nc.sync.dma_start(wat[:], p_wa.ap()[:, r * 128:(r + 1) * 128])
            ap_ = pp_mm.tile([128, TH], F32, tag="mm")
            nc.tensor.matmul(ap_[:], wat[:], actT[:], start=True, stop=True)
            pt = tmp.tile([128, TH], F32, tag="t32")
            nc.sync.dma_start(pt[:], p_posT.ap()[(4 + r) * 128:(5 + r) * 128, :])
            nc.vector.tensor_tensor(h[:, 4 + r, :], ap_[:], pt[:], AluOp.add)

        # ---------------- transformer layers ----------------
        for l in range(NL):
            z1 = zpool.tile([128, DCH, TH], BF, tag="z", bufs=2)
            layernorm(z1)

            # AllGather z^T across the core pair -> z for both halves.
            zin = dram.tile([D, TH], BF, tag="zin")
            for d in range(DCH):
                nc.sync.dma_start(zin[d * 128:(d + 1) * 128, :], z1[:, d, :])
            zout = dram.tile([2 * D, TH], BF, tag="zout")
            nc.gpsimd.collective_compute(
                "AllGather",
                AluOp.bypass,
                replica_groups=[[0, 1], [2, 3], [4, 5], [6, 7]],
                ins=[zin.opt()],
                outs=[zout.opt()],
            )
            zfull = zpool.tile([128, 2 * DCH, TH], BF, tag="zfull")
            nc.sync.dma_start(zfull[:],
                              zout.rearrange("(c p) t -> p c t", p=128))

            # ---- Q^T (own tokens) ----
            wq3 = p_wq.ap()[l].rearrange("(c p) n -> p c n", p=128)
            for r in range(DCH):
                wt = wpool.tile([128, DCH, 128], BF, tag="w")
                nc.sync.dma_start(wt[:], wq3[:, :, r * 128:(r + 1) * 128])
                qp = pp_mm.tile([128, TH], F32, tag="mm")
                for d in range(DCH):
                    nc.tensor.matmul(qp[:], wt[:, d, :], z1[:, d, :],
                                     start=(d == 0), stop=(d == DCH - 1))
                nc.scalar.copy(QT[:, r, :], qp[:])

            # ---- K^T (all tokens) ----
            wk3 = p_wk.ap()[l].rearrange("(c p) n -> p c n", p=128)
            for r in range(DCH):
                wt = wpool.tile([128, DCH, 128], BF, tag="w")
                nc.sync.dma_start(wt[:], wk3[:, :, r * 128:(r + 1) * 128])
                for hh in range(2):
                    kp = pp_mm.tile([128, TH], F32, tag="mm")
                    for d in range(DCH):
                        nc.tensor.matmul(kp[:], wt[:, d, :],
                                         zfull[:, hh * DCH + d, :],
                                         start=(d == 0), stop=(d == DCH - 1))
                    nc.scalar.copy(KT[:, r, hh * TH:(hh + 1) * TH], kp[:])

            # ---- V (all tokens, T-major, 65-wide head blocks w/ ones col) ----
            for c in range(8):  # 8 token chunks of 128
                nc.gpsimd.memset(
                    VA[:, c, :].rearrange("p (hd w) -> p hd w", w=65)[:, :, 64:65],
                    1.0)
            wv3 = p_wv.ap()[l].rearrange("(c p) n -> p c n", p=128)
            for nn in range(2):
                wvt = wvpool.tile([128, DCH, 512], BF, tag="wv8")
                nc.sync.dma_start(wvt[:], wv3[:, :, nn * 512:(nn + 1) * 512])
                for c in range(8):  # token chunk (hh*4 + tb)
                    hh, tb = c // 4, c % 4
                    vp = pp_mm.tile([128, 512], F32, tag="mm")
                    for d in range(DCH):
                        nc.tensor.matmul(
                            vp[:],
                            zfull[:, hh * DCH + d, tb * 128:(tb + 1) * 128],
                            wvt[:, d, :],
                            start=(d == 0), stop=(d == DCH - 1))
                    # scatter 8 heads of 64 into the 65-wide blocks
                    nc.vector.tensor_copy(
                        VA[:, c, nn * 8 * 65:(nn * 8 + 8) * 65].rearrange(
                            "p (hd w) -> p hd w", w=65)[:, :, 0:64],
                        vp.rearrange("p (hd w) -> p hd w", w=64),
                    )

            # ---- attention ----
            for hd in range(NH):
                rc, ro = hd // 2, (hd % 2) * 64
                o_p = pp_o.tile([65, TH], F32, tag="o")
                for kc in range(8):
                    s_p = pp_s.tile([128, TH], F32, tag="s")
                    nc.tensor.matmul(
                        s_p[:],
                        KT[ro:ro + 64, rc, kc * 128:(kc + 1) * 128],
                        QT[ro:ro + 64, rc, :],
                        start=True, stop=True)
                    nc.vector.tensor_tensor(s_p[:], s_p[:], maskT[:, kc, :],
                                            AluOp.add)
                    p_t = ptp.tile([128, TH], BF, tag="pt")
                    nc.scalar.activation(p_t[:], s_p[:], Act.Exp,
                                         scale=1.0 / float(np.sqrt(DH)))
                    nc.tensor.matmul(
                        o_p[:],
                        VA[:, kc, hd * 65:(hd + 1) * 65],
                        p_t[:],
                        start=(kc == 0), stop=(kc == 7))
                inv = stat.tile([1, TH], F32, tag="stat")
                nc.vector.reciprocal(inv[:], o_p[64:65, :])
                ivB = pp_ln.tile([64, TH], F32, tag="bcast", bufs=2)
                nc.tensor.matmul(ivB[:], ones_row[0:1, 0:64], inv[:],
                                 start=True, stop=True)
                ivS = tmp.tile([64, TH], F32, tag="ivs")
                nc.scalar.copy(ivS[:], ivB[:])
                nc.vector.tensor_tensor(yT[ro:ro + 64, rc, :], o_p[0:64, :],
                                        ivS[:], AluOp.mult)

            # ---- proj + residual ----
            wp3 = p_wp.ap()[l].rearrange("(c p) n -> p c n", p=128)
            for r in range(DCH):
                wt = wpool.tile([128, DCH, 128], BF, tag="w")
                nc.sync.dma_start(wt[:], wp3[:, :, r * 128:(r + 1) * 128])
                pp = pp_mm.tile([128, TH], F32, tag="mm")
                for d in range(DCH):
                    nc.tensor.matmul(pp[:], wt[:, d, :], yT[:, d, :],
                                     start=(d == 0), stop=(d == DCH - 1))
                nc.vector.tensor_tensor(h[:, r, :], h[:, r, :], pp[:],
                                        AluOp.add)

            # ---- MLP ----
            z2 = zpool.tile([128, DCH, TH], BF, tag="z", bufs=2)
            layernorm(z2)
            aT = big.tile([128, 32, TH], BF, tag="aT")
            w13 = p_w1.ap()[l].rearrange("(c p) f -> p c f", p=128)
            for ft in range(32):
                w1t = wpool.tile([128, DCH, 128], BF, tag="w")
                nc.sync.dma_start(w1t[:], w13[:, :, ft * 128:(ft + 1) * 128])
                fp = pp_mm.tile([128, TH], F32, tag="mm")
                for d in range(DCH):
                    nc.tensor.matmul(fp[:], w1t[:, d, :], z2[:, d, :],
                                     start=(d == 0), stop=(d == DCH - 1))
                nc.scalar.activation(aT[:, ft, :], fp[:], Act.Gelu)
            w23 = p_w2.ap()[l].rearrange("(c p) n -> p c n", p=128)
            for r in range(DCH):
                w2t = wvpool.tile([128, FF // 128, 128], BF, tag="wv8")
                nc.sync.dma_start(w2t[:], w23[:, :, r * 128:(r + 1) * 128])
                mp = pp_mm.tile([128, TH], F32, tag="mm")
                for fc in range(32):
                    nc.tensor.matmul(mp[:], w2t[:, fc, :], aT[:, fc, :],
                                     start=(fc == 0), stop=(fc == 31))
                nc.vector.tensor_tensor(h[:, r, :], h[:, r, :], mp[:],
                                        AluOp.add)

        # ---------------- final LN + head ----------------
        zf = zpool.tile([128, DCH, TH], BF, tag="z", bufs=2)
        layernorm(zf)
        wht = wvpool.tile([128, DCH, E], BF, tag="wv8")
        nc.sync.dma_start(wht[:], p_wh.ap().rearrange("(c p) e -> p c e", p=128))
        for tb in range(TCH):
            op_ = pp_mm.tile([128, E], F32, tag="mm")
            for d in range(DCH):
                nc.tensor.matmul(
                    op_[:],
                    zf[:, d, tb * 128:(tb + 1) * 128],
                    wht[:, d, :],
                    start=(d == 0), stop=(d == DCH - 1))
            ot = tmp.tile([128, E], F32, tag="t32")
            nc.scalar.copy(ot[:], op_[:])
            nc.sync.dma_start(p_out.ap()[tb * 128:(tb + 1) * 128, :], ot[:])

        for _pool in reversed((const, persist, zpool, big, wpool, wvpool, tmp,
                               stat, ptp, dram, pp_mm, pp_s, pp_o, pp_ln)):
            _pool.release()

    nc.compile()
    return nc


def _get_program():
    if "nc" not in _cache:
        _cache["nc"] = _build_program()
    return _cache["nc"]


def _bf16(x):
    return np.ascontiguousarray(np.asarray(x).astype(ml_dtypes.bfloat16))


def _f32(x):
    return np.ascontiguousarray(np.asarray(x).astype(np.float32))


def make_in_maps(inputs):
    lcd = np.asarray(inputs["lcd"], np.float32).reshape(B, T, E)
    lcd_shift = np.concatenate(
        [np.zeros((B, 1, E), np.float32), lcd[:, :-1]], axis=1)
    action = np.asarray(inputs["action"], np.float32)
    pos = np.asarray(inputs["pos_emb"], np.float32)[0]          # [T, D]

    shared = {
        "W_embed": _bf16(inputs["W_embed"]),
        "W_act": _f32(inputs["W_act"]),
        "Wq": _bf16(inputs["Wq"]),
        "Wk": _bf16(inputs["Wk"]),
        "Wv": _bf16(inputs["Wv"]),
        "Wp": _bf16(inputs["Wp"]),
        "W1": _bf16(inputs["W1"]),
        "W2": _bf16(inputs["W2"]),
        "Wh": _bf16(inputs["Wh"]),
    }

    in_maps = []
    for c in range(NC):
        b, half = c // 2, c % 2
        tok = np.arange(half * TH, (half + 1) * TH)             # abs own tokens
        kabs = np.arange(T)                                     # abs key index
        # additive causal mask in S^T layout: [k-block, 128 k, TH q]
        m = np.where(kabs[:, None] <= tok[None, :], 0.0, -1e9).astype(np.float32)
        maskT = m.reshape(8, 128, TH)
        in_maps.append(dict(
            shared,
            lcdT=_bf16(lcd_shift[b, tok].T),                    # [E, TH]
            actT=_f32(action[b, tok].T),                        # [AD, TH]
            posT=_f32(pos[tok].T),                              # [D, TH]
            maskT=_bf16(maskT),
        ))
    return in_maps


def assemble(results):
    out = np.empty((B, T, E), np.float32)
    for c in range(NC):
        b, half = c // 2, c % 2
        out[b, half * TH:(half + 1) * TH] = results[c]["out"]
    return out


def kernel(**inputs):
    nc = _get_program()
    in_maps = make_in_maps(inputs)
    res = run_bass_kernel_spmd(nc, in_maps, list(range(NC)))
    return assemble(res.results)



# revision 24
# speedup vs baseline: 1.0403x; 1.0403x over previous
"""GPT forward pass on 8 TRN2 NeuronCores.

Sharding: core c -> batch b = c // 2, sequence half = c % 2 (contiguous
512-token halves).  The residual stream stays core-local in a D-major
layout (h^T: [D=1024 partition-chunks, 512 own tokens]).  Each core
computes K/V only for its own 512 tokens; per layer two AllGathers over
the core pair exchange K^T (bf16, [D, TH]) and V (bf16, [TH, D]) so both
cores see the full-sequence K/V.

Attention uses the S^T scheme: S^T = K_h @ Q_h^T per 128-key block.  Q is
stored zero-padded per head ([128, NH, TH] with the other head's 64 rows
zeroed) so every S matmul has a full 128-row contraction and 128-wide
stationary operand (fast-weight-load path).  The PV stationary is
[V_h | ones] (128 wide): output rows 0:64 are y_h, rows 64:128 are the
softmax denominator already broadcast.  Masking is a post-exp multiply
with a 0/1 bf16 mask; denominators are batched into one
reciprocal_approx_fast per layer.

LayerNorm statistics are computed with a [128,128] ones matmul in
float32r (broadcast sums), variance on the vector engine, and
1/sqrt via reciprocal_approx_fast + scalar sqrt.

Weights are pre-cast to bf16 on the host; matmuls accumulate in f32 PSUM;
the residual stream stays f32.  Biases and LN affine params are zeros/ones
in this model and are skipped.
"""

import sys

sys.path.insert(0, "/opt/trn_rl_repo")

import numpy as np
import ml_dtypes

import concourse.bass as bass
import concourse.bacc as bacc
import concourse.mybir as mybir
from concourse import tile
from concourse.bass_utils import run_bass_kernel_spmd

B, T, E, D, NH, DH, NL, FF, AD = 4, 1024, 512, 1024, 16, 64, 8, 4096, 8
TH = T // 2          # tokens per core
NC = 8
DCH = D // 128       # 8 partition chunks of the embedding dim
TCH = TH // 128      # 4 token tiles per half
EPS = 1e-5
BF = mybir.dt.bfloat16
F32 = mybir.dt.float32
F32R = mybir.dt.float32r
AluOp = mybir.AluOpType
Act = mybir.ActivationFunctionType

_cache = {}


def _build_program():
    nc = bacc.Bacc("TRN2", target_bir_lowering=False, debug=False, num_devices=NC)

    # --- DRAM parameters (identical graph on all cores; data differs) ---
    p_lcdT = nc.declare_dram_parameter("lcdT", [E, TH], BF, isOutput=False)
    p_actT = nc.declare_dram_parameter("actT", [AD, TH], F32, isOutput=False)
    p_posT = nc.declare_dram_parameter("posT", [D, TH], F32, isOutput=False)
    p_we = nc.declare_dram_parameter("W_embed", [E, D // 2], BF, isOutput=False)
    p_wa = nc.declare_dram_parameter("W_act", [AD, D // 2], F32, isOutput=False)
    p_wq = nc.declare_dram_parameter("Wq", [NL, D, D], BF, isOutput=False)
    p_wk = nc.declare_dram_parameter("Wk", [NL, D, D], BF, isOutput=False)
    p_wv = nc.declare_dram_parameter("Wv", [NL, D, D], BF, isOutput=False)
    p_wp = nc.declare_dram_parameter("Wp", [NL, D, D], BF, isOutput=False)
    p_w1 = nc.declare_dram_parameter("W1", [NL, D, FF], BF, isOutput=False)
    p_w2 = nc.declare_dram_parameter("W2", [NL, FF, D], BF, isOutput=False)
    p_wh = nc.declare_dram_parameter("Wh", [D, E], BF, isOutput=False)
    p_mask = nc.declare_dram_parameter("mask01", [8, 128, TH], BF, isOutput=False)
    p_ones = nc.declare_dram_parameter("ones128", [128, 128], F32R, isOutput=False)
    p_out = nc.declare_dram_parameter("out", [TH, E], F32, isOutput=True)

    with tile.TileContext(nc) as tc:
        # ---------------- pools ----------------
        const = tc.alloc_tile_pool(name="const", bufs=1)
        persist = tc.alloc_tile_pool(name="persist", bufs=1)
        zpool = tc.alloc_tile_pool(name="zpool", bufs=1)
        big = tc.alloc_tile_pool(name="bigact", bufs=1)
        wpool = tc.alloc_tile_pool(name="wpool", bufs=3)
        wvpool = tc.alloc_tile_pool(name="wvpool", bufs=2)
        stage = tc.alloc_tile_pool(name="stage", bufs=1)
        tmp = tc.alloc_tile_pool(name="tmp", bufs=3)
        stat = tc.alloc_tile_pool(name="stat", bufs=3)
        ptp = tc.alloc_tile_pool(name="ptp", bufs=3)
        dram = tc.alloc_tile_pool(name="dram", bufs=2, space="DRAM")
        pp_mm = tc.alloc_tile_pool(name="pp_mm", bufs=2, space="PSUM")
        pp_pair = tc.alloc_tile_pool(name="pp_pair", bufs=2, space="PSUM")
        pp_o = tc.alloc_tile_pool(name="pp_o", bufs=2, space="PSUM")

        ones128 = const.tile([128, 128], F32R)
        nc.sync.dma_start(ones128[:], p_ones.ap())

        # residual stream h^T, f32, D-chunk d at [:, d, :]
        h = persist.tile([128, DCH, TH], F32R)
        # multiplicative 0/1 causal mask in S^T layout, k-block kc at [:, kc, :]
        mask01 = persist.tile([128, 8, TH], BF)
        nc.sync.dma_start(mask01[:], p_mask.ap().rearrange("k p t -> p k t"))

        # Q^T zero-padded per head: head hd lives in rows (hd%2)*64.. of
        # [:, hd, :]; the other 64 rows stay zero so S matmuls contract 128.
        QT2 = persist.tile([128, NH, TH], BF)
        nc.gpsimd.memset(QT2[:], 0.0)
        KT = persist.tile([128, DCH, T], BF)     # K^T rows=D, cols=all tok
        # V rows=tok, 65-wide head blocks [V_h (64) | ones (1)], padded so the
        # 128-wide PV stationary slice [hd*65 : hd*65+128] is in bounds (the
        # trailing 63 cols of the slice produce junk output rows 65:128 that
        # are never read).
        VA2 = persist.tile([128, 8, NH * 65 + 63], BF)
        for hd in range(NH):
            nc.gpsimd.memset(VA2[:, :, hd * 65 + 64:hd * 65 + 65], 1.0)
        yT = persist.tile([128, DCH, TH], BF)    # attn out^T, rows=D

        # ---------------- helpers ----------------
        def layernorm(z_out, src):
            """z_out (sbuf bf16 [128, DCH, TH]) = LayerNorm(src) in D-major."""
            s_b = pp_mm.tile([128, TH], F32, tag="mm")
            for d in range(DCH):
                nc.tensor.matmul(s_b[:], ones128[:], src[:, d, :],
                                 start=(d == 0), stop=(d == DCH - 1))
            q_b = pp_mm.tile([128, TH], F32, tag="mm")
            for d in range(DCH):
                sq = tmp.tile([128, TH], F32R, tag="sq", bufs=2)
                nc.scalar.square(sq[:], src[:, d, :])
                nc.tensor.matmul(q_b[:], ones128[:], sq[:],
                                 start=(d == 0), stop=(d == DCH - 1))
            mean = stat.tile([128, TH], F32, tag="stat")
            nc.vector.tensor_scalar_mul(mean[:], s_b[:], 1.0 / D)
            a = stat.tile([128, TH], F32, tag="stat")
            nc.vector.tensor_tensor(a[:], mean[:], mean[:], AluOp.mult)
            nc.vector.scalar_tensor_tensor(a[:], q_b[:], 1.0 / D, a[:],
                                           AluOp.mult, AluOp.subtract)
            nc.vector.tensor_scalar_add(a[:], a[:], EPS)
            nc.vector.reciprocal_approx_fast(a[:], a[:])
            rinv = stat.tile([128, TH], F32, tag="stat")
            nc.scalar.sqrt(rinv[:], a[:])
            mr = mean
            nc.vector.tensor_tensor(mr[:], mean[:], rinv[:], AluOp.mult)
            for d in range(DCH):
                eng = nc.vector if d < 4 else nc.gpsimd
                t = tmp.tile([128, TH], F32, tag="zt", bufs=2)
                eng.tensor_tensor(t[:], src[:, d, :], rinv[:], AluOp.mult)
                eng.tensor_tensor(z_out[:, d, :], t[:], mr[:], AluOp.subtract)

        # ---------------- embedding ----------------
        we3 = p_we.ap().rearrange("(c p) n -> p c n", p=128)      # [128, 4, 512]
        for r in range(4):
            wet = tmp.tile([128, 4, 128], BF, tag="tbf", bufs=2)
            nc.sync.dma_start(wet[:], we3[:, :, r * 128:(r + 1) * 128])
            ep = pp_mm.tile([128, TH], F32, tag="mm")
            for ec in range(4):
                lt = tmp.tile([128, TH], BF, tag="tbf", bufs=2)
                nc.sync.dma_start(lt[:], p_lcdT.ap()[ec * 128:(ec + 1) * 128, :])
                nc.tensor.matmul(ep[:], wet[:, ec, :], lt[:],
                                 start=(ec == 0), stop=(ec == 3))
            pt = tmp.tile([128, TH], F32, tag="t32", bufs=2)
            nc.sync.dma_start(pt[:], p_posT.ap()[r * 128:(r + 1) * 128, :])
            nc.vector.tensor_tensor(h[:, r, :], ep[:], pt[:], AluOp.add)
        actT = tmp.tile([AD, TH], F32, tag="t32", bufs=2)
        nc.sync.dma_start(actT[:], p_actT.ap())
        for r in range(4):
            wat = tmp.tile([AD, 128], F32, tag="t32", bufs=2)
            nc.sync.dma_start(wat[:], p_wa.ap()[:, r * 128:(r + 1) * 128])
            ap_ = pp_mm.tile([128, TH], F32, tag="mm")
            nc.tensor.matmul(ap_[:], wat[:], actT[:], start=True, stop=True)
            pt = tmp.tile([128, TH], F32, tag="t32", bufs=2)
            nc.sync.dma_start(pt[:], p_posT.ap()[(4 + r) * 128:(5 + r) * 128, :])
            nc.vector.tensor_tensor(h[:, 4 + r, :], ap_[:], pt[:], AluOp.add)

        # ---------------- transformer layers ----------------
        for l in range(NL):
            z1 = zpool.tile([128, DCH, TH], BF, tag="z", bufs=2)
            layernorm(z1, h)

            # ---- K^T own tokens, then AllGather over the pair ----
            wk3 = p_wk.ap()[l].rearrange("(c p) n -> p c n", p=128)
            kloc = stage.tile([128, DCH, TH], BF, tag="kloc")
            for r in range(DCH):
                wt = wpool.tile([128, DCH, 128], BF, tag="w")
                nc.sync.dma_start(wt[:], wk3[:, :, r * 128:(r + 1) * 128])
                kp = pp_mm.tile([128, TH], F32, tag="mm")
                for d in range(DCH):
                    nc.tensor.matmul(kp[:], wt[:, d, :], z1[:, d, :],
                                     start=(d == 0), stop=(d == DCH - 1))
                nc.vector.tensor_copy(kloc[:, r, :], kp[:])
            kin = dram.tile([D, TH], BF, tag="kin")
            nc.sync.dma_start(kin.rearrange("(r p) t -> p r t", p=128), kloc[:])
            kout = dram.tile([2 * D, TH], BF, tag="kout")
            nc.gpsimd.collective_compute(
                "AllGather", AluOp.bypass,
                replica_groups=[[0, 1], [2, 3], [4, 5], [6, 7]],
                ins=[kin.opt()], outs=[kout.opt()])

            # ---- V own tokens (token-major), then AllGather ----
            wv3 = p_wv.ap()[l].rearrange("(c p) n -> p c n", p=128)
            vloc = stage.tile([128, TCH, D], BF, tag="vloc")
            for nn in range(2):
                wvt = wvpool.tile([128, DCH, 512], BF, tag="wv8")
                nc.sync.dma_start(wvt[:], wv3[:, :, nn * 512:(nn + 1) * 512])
                for tb in range(TCH):
                    vp = pp_mm.tile([128, 512], F32, tag="mm")
                    for d in range(DCH):
                        nc.tensor.matmul(
                            vp[:], z1[:, d, tb * 128:(tb + 1) * 128],
                            wvt[:, d, :],
                            start=(d == 0), stop=(d == DCH - 1))
                    nc.vector.tensor_copy(vloc[:, tb, nn * 512:(nn + 1) * 512],
                                          vp[:])
            vin = dram.tile([TH, D], BF, tag="vin")
            nc.sync.dma_start(vin.rearrange("(c p) d -> p c d", p=128), vloc[:])
            vout = dram.tile([T, D], BF, tag="vout")
            nc.gpsimd.collective_compute(
                "AllGather", AluOp.bypass,
                replica_groups=[[0, 1], [2, 3], [4, 5], [6, 7]],
                ins=[vin.opt()], outs=[vout.opt()])

            # ---- Q^T own tokens into zero-padded per-head slots ----
            wq3 = p_wq.ap()[l].rearrange("(c p) n -> p c n", p=128)
            for r in range(DCH):
                wt = wpool.tile([128, DCH, 128], BF, tag="w")
                nc.sync.dma_start(wt[:], wq3[:, :, r * 128:(r + 1) * 128])
                qp = pp_mm.tile([128, TH], F32, tag="mm")
                for d in range(DCH):
                    nc.tensor.matmul(qp[:], wt[:, d, :], z1[:, d, :],
                                     start=(d == 0), stop=(d == DCH - 1))
                nc.vector.tensor_copy(QT2[0:64, 2 * r, :], qp[0:64, :])
                nc.vector.tensor_copy(QT2[64:128, 2 * r + 1, :], qp[64:128, :])

            # ---- land gathered K into KT (absolute token order) ----
            ko3 = kout.rearrange("(hh r p) t -> r p hh t", p=128, hh=2)
            for r in range(DCH):
                nc.sync.dma_start(
                    KT[:, r, :].rearrange("p (hh t) -> p hh t", hh=2),
                    ko3[r])
            # ---- land gathered V into VA2 (skip the ones columns) ----
            vo3 = vout.rearrange("(c p) (hd dh) -> hd p c dh", p=128, dh=DH)
            for hd in range(NH):
                nc.sync.dma_start(VA2[:, :, hd * 65:hd * 65 + 64], vo3[hd])

            # ---- attention ----
            for hd in range(NH):
                rc, ro = hd // 2, (hd % 2) * 64
                o_p = pp_o.tile([128, TH], F32, tag="o")
                for kcp in range(4):
                    s2 = pp_pair.tile([128, 2, TH], F32, tag="pair")
                    for j in range(2):
                        kc = kcp * 2 + j
                        nc.tensor.matmul(
                            s2[:, j, :],
                            KT[:, rc, kc * 128:(kc + 1) * 128],
                            QT2[:, hd, :],
                            start=True, stop=True)
                    p2 = ptp.tile([128, 2, TH], BF, tag="p2", bufs=2)
                    nc.scalar.activation(p2[:], s2[:], Act.Exp,
                                         scale=1.0 / float(np.sqrt(DH)))
                    eng = nc.vector if hd % 2 == 0 else nc.gpsimd
                    eng.tensor_tensor(p2[:], p2[:],
                                      mask01[:, 2 * kcp:2 * kcp + 2, :],
                                      AluOp.mult)
                    for j in range(2):
                        kc = kcp * 2 + j
                        nc.tensor.matmul(
                            o_p[:], VA2[:, kc, hd * 65:hd * 65 + 128],
                            p2[:, j, :],
                            start=(kcp == 0 and j == 0),
                            stop=(kcp == 3 and j == 1))
                # normalize: row 64 of o_p is the denominator; invert it,
                # replicate to 64 partitions with a 0-stride DMA, then one
                # multiply writes yT.
                inv1 = stat.tile([1, TH], F32, tag="den1", bufs=2)
                nc.vector.tensor_copy(inv1[:], o_p[64:65, :])
                nc.vector.reciprocal_approx_fast(inv1[:], inv1[:])
                invO = stat.tile([64, TH], F32, tag="den", bufs=2)
                nc.gpsimd.partition_broadcast(invO[:], inv1[0:1, :])
                nc.vector.tensor_tensor(yT[ro:ro + 64, rc, :], o_p[0:64, :],
                                        invO[:], AluOp.mult)

            # ---- proj + residual ----
            wp3 = p_wp.ap()[l].rearrange("(c p) n -> p c n", p=128)
            for r in range(DCH):
                wt = wpool.tile([128, DCH, 128], BF, tag="w")
                nc.sync.dma_start(wt[:], wp3[:, :, r * 128:(r + 1) * 128])
                pp = pp_mm.tile([128, TH], F32, tag="mm")
                for d in range(DCH):
                    nc.tensor.matmul(pp[:], wt[:, d, :], yT[:, d, :],
                                     start=(d == 0), stop=(d == DCH - 1))
                nc.vector.tensor_tensor(h[:, r, :], h[:, r, :], pp[:],
                                        AluOp.add)

            # ---- MLP ----
            z2 = zpool.tile([128, DCH, TH], BF, tag="z", bufs=2)
            layernorm(z2, h)
            aT = big.tile([128, 32, TH], BF, tag="aT")
            w13 = p_w1.ap()[l].rearrange("(c p) f -> p c f", p=128)
            for fi in range(16):
                w1t = wpool.tile([128, DCH, 256], BF, tag="w1p", bufs=2)
                nc.sync.dma_start(w1t[:], w13[:, :, fi * 256:(fi + 1) * 256])
                fp = pp_pair.tile([128, 2, TH], F32, tag="pair")
                for j in range(2):
                    for d in range(DCH):
                        nc.tensor.matmul(
                            fp[:, j, :], w1t[:, d, j * 128:(j + 1) * 128],
                            z2[:, d, :],
                            start=(d == 0), stop=(d == DCH - 1))
                nc.scalar.activation(aT[:, 2 * fi:2 * fi + 2, :], fp[:],
                                     Act.Gelu)
            w23 = p_w2.ap()[l].rearrange("(c p) n -> p c n", p=128)
            for r in range(DCH):
                w2t = wvpool.tile([128, FF // 128, 128], BF, tag="wv8")
                nc.sync.dma_start(w2t[:], w23[:, :, r * 128:(r + 1) * 128])
                mp = pp_mm.tile([128, TH], F32, tag="mm")
                for fc in range(32):
                    nc.tensor.matmul(mp[:], w2t[:, fc, :], aT[:, fc, :],
                                     start=(fc == 0), stop=(fc == 31))
                nc.vector.tensor_tensor(h[:, r, :], h[:, r, :], mp[:],
                                        AluOp.add)

        # ---------------- final LN + head ----------------
        zf = zpool.tile([128, DCH, TH], BF, tag="z", bufs=2)
        layernorm(zf, h)
        wht = wvpool.tile([128, DCH, E], BF, tag="wv8")
        nc.sync.dma_start(wht[:], p_wh.ap().rearrange("(c p) e -> p c e", p=128))
        for tb in range(TCH):
            op_ = pp_mm.tile([128, E], F32, tag="mm")
            for d in range(DCH):
                nc.tensor.matmul(
                    op_[:],
                    zf[:, d, tb * 128:(tb + 1) * 128],
                    wht[:, d, :],
                    start=(d == 0), stop=(d == DCH - 1))
            ot = tmp.tile([128, E], F32, tag="t32", bufs=2)
            nc.scalar.copy(ot[:], op_[:])
            nc.sync.dma_start(p_out.ap()[tb * 128:(tb + 1) * 128, :], ot[:])

        for _pool in reversed((const, persist, zpool, big, wpool, wvpool,
                               stage, tmp, stat, ptp, dram, pp_mm, pp_pair,
                               pp_o)):
            _pool.release()

    nc.compile()
    return nc


def _get_program():
    if "nc" not in _cache:
        _cache["nc"] = _build_program()
    return _cache["nc"]


def _bf16(x):
    return np.ascontiguousarray(np.asarray(x).astype(ml_dtypes.bfloat16))


def _f32(x):
    return np.ascontiguousarray(np.asarray(x).astype(np.float32))


def make_in_maps(inputs):
    lcd = np.asarray(inputs["lcd"], np.float32).reshape(B, T, E)
    lcd_shift = np.concatenate(
        [np.zeros((B, 1, E), np.float32), lcd[:, :-1]], axis=1)
    action = np.asarray(inputs["action"], np.float32)
    pos = np.asarray(inputs["pos_emb"], np.float32)[0]          # [T, D]

    shared = {
        "W_embed": _bf16(inputs["W_embed"]),
        "W_act": _f32(inputs["W_act"]),
        "Wq": _bf16(inputs["Wq"]),
        "Wk": _bf16(inputs["Wk"]),
        "Wv": _bf16(inputs["Wv"]),
        "Wp": _bf16(inputs["Wp"]),
        "W1": _bf16(inputs["W1"]),
        "W2": _bf16(inputs["W2"]),
        "Wh": _bf16(inputs["Wh"]),
        "ones128": np.ones((128, 128), np.float32),
    }

    in_maps = []
    for c in range(NC):
        b, half = c // 2, c % 2
        tok = np.arange(half * TH, (half + 1) * TH)             # abs own tokens
        kabs = np.arange(T)                                     # abs key index
        # multiplicative 0/1 causal mask in S^T layout: [k-block, 128 k, TH q]
        m = (kabs[:, None] <= tok[None, :]).astype(np.float32)
        mask01 = m.reshape(8, 128, TH)
        in_maps.append(dict(
            shared,
            lcdT=_bf16(lcd_shift[b, tok].T),                    # [E, TH]
            actT=_f32(action[b, tok].T),                        # [AD, TH]
            posT=_f32(pos[tok].T),                              # [D, TH]
            mask01=_bf16(mask01),
        ))
    return in_maps


def assemble(results):
    out = np.empty((B, T, E), np.float32)
    for c in range(NC):
        b, half = c // 2, c % 2
        out[b, half * TH:(half + 1) * TH] = results[c]["out"]
    return out


def kernel(**inputs):
    nc = _get_program()
    in_maps = make_in_maps(inputs)
    res = run_bass_kernel_spmd(nc, in_maps, list(range(NC)))
    return assemble(res.results)


# revision 27
# speedup vs baseline: 1.6571x; 1.5929x over previous
"""GPT forward pass on 8 TRN2 NeuronCores.

Sharding: core c -> batch b = c // 2, sequence half = c % 2 (contiguous
512-token halves).  The residual stream stays core-local in a D-major
layout (h^T: [D=1024 partition-chunks, 512 own tokens]).  Each core
computes K/V only for its own 512 tokens; per layer two AllGathers over
the core pair exchange K^T (bf16, [D, TH]) and V (bf16, [TH, D]) so both
cores see the full-sequence K/V.

Attention uses the S^T scheme: S^T = K_h @ Q_h^T per 128-key block.  Q is
stored zero-padded per head ([128, NH, TH] with the other head's 64 rows
zeroed) so every S matmul has a full 128-row contraction and 128-wide
stationary operand (fast-weight-load path).  The PV stationary is
[V_h | ones] (128 wide): output rows 0:64 are y_h, rows 64:128 are the
softmax denominator already broadcast.  Masking is a post-exp multiply
with a 0/1 bf16 mask; denominators are batched into one
reciprocal_approx_fast per layer.

LayerNorm statistics are computed with a [128,128] ones matmul in
float32r (broadcast sums), variance on the vector engine, and
1/sqrt via reciprocal_approx_fast + scalar sqrt.

Weights are pre-cast to bf16 on the host; matmuls accumulate in f32 PSUM;
the residual stream stays f32.  Biases and LN affine params are zeros/ones
in this model and are skipped.
"""

import sys

sys.path.insert(0, "/opt/trn_rl_repo")

import numpy as np
import ml_dtypes

import concourse.bass as bass
import concourse.bacc as bacc
import concourse.mybir as mybir
from concourse import tile
from concourse.bass_utils import run_bass_kernel_spmd

B, T, E, D, NH, DH, NL, FF, AD = 4, 1024, 512, 1024, 16, 64, 8, 4096, 8
TH = T // 2          # tokens per core
NC = 8
DCH = D // 128       # 8 partition chunks of the embedding dim
TCH = TH // 128      # 4 token tiles per half
EPS = 1e-5
BF = mybir.dt.bfloat16
F32 = mybir.dt.float32
F32R = mybir.dt.float32r
AluOp = mybir.AluOpType
Act = mybir.ActivationFunctionType

_cache = {}


def _build_program():
    nc = bacc.Bacc("TRN2", target_bir_lowering=False, debug=False, num_devices=NC)

    # --- DRAM parameters (identical graph on all cores; data differs) ---
    p_lcdT = nc.declare_dram_parameter("lcdT", [E, TH], BF, isOutput=False)
    p_actT = nc.declare_dram_parameter("actT", [AD, TH], F32, isOutput=False)
    p_posT = nc.declare_dram_parameter("posT", [D, TH], F32, isOutput=False)
    p_we = nc.declare_dram_parameter("W_embed", [E, D // 2], BF, isOutput=False)
    p_wa = nc.declare_dram_parameter("W_act", [AD, D // 2], F32, isOutput=False)
    p_wq = nc.declare_dram_parameter("Wq", [NL, D, D], BF, isOutput=False)
    p_wk = nc.declare_dram_parameter("Wk", [NL, D, D], BF, isOutput=False)
    p_wv = nc.declare_dram_parameter("Wv", [NL, D, D], BF, isOutput=False)
    p_wp = nc.declare_dram_parameter("Wp", [NL, D, D], BF, isOutput=False)
    p_w1 = nc.declare_dram_parameter("W1", [NL, D, FF], BF, isOutput=False)
    p_w2 = nc.declare_dram_parameter("W2", [NL, FF, D], BF, isOutput=False)
    p_wh = nc.declare_dram_parameter("Wh", [D, E], BF, isOutput=False)
    p_mask = nc.declare_dram_parameter("mask01", [8, 128, TH], BF, isOutput=False)
    p_ones = nc.declare_dram_parameter("ones128", [128, 128], F32R, isOutput=False)
    p_out = nc.declare_dram_parameter("out", [TH, E], F32, isOutput=True)

    with tile.TileContext(nc) as tc:
        # ---------------- pools ----------------
        const = tc.alloc_tile_pool(name="const", bufs=1)
        persist = tc.alloc_tile_pool(name="persist", bufs=1)
        zpool = tc.alloc_tile_pool(name="zpool", bufs=1)
        big = tc.alloc_tile_pool(name="bigact", bufs=1)
        wpool = tc.alloc_tile_pool(name="wpool", bufs=3)
        wvpool = tc.alloc_tile_pool(name="wvpool", bufs=2)
        stage = tc.alloc_tile_pool(name="stage", bufs=1)
        tmp = tc.alloc_tile_pool(name="tmp", bufs=3)
        stat = tc.alloc_tile_pool(name="stat", bufs=3)
        ptp = tc.alloc_tile_pool(name="ptp", bufs=3)
        dram = tc.alloc_tile_pool(name="dram", bufs=2, space="DRAM")
        pp_mm = tc.alloc_tile_pool(name="pp_mm", bufs=2, space="PSUM")
        pp_pair = tc.alloc_tile_pool(name="pp_pair", bufs=2, space="PSUM")
        pp_o = tc.alloc_tile_pool(name="pp_o", bufs=2, space="PSUM")

        ones128 = const.tile([128, 128], F32R)
        nc.sync.dma_start(ones128[:], p_ones.ap())
        eps_t = const.tile([128, 1], F32)
        nc.gpsimd.memset(eps_t[:], EPS)

        # residual stream h^T, f32, D-chunk d at [:, d, :]
        h = persist.tile([128, DCH, TH], F32R)
        # multiplicative 0/1 causal mask in S^T layout, k-block kc at [:, kc, :]
        mask01 = persist.tile([128, 8, TH], BF)
        nc.sync.dma_start(mask01[:], p_mask.ap().rearrange("k p t -> p k t"))

        # Q^T zero-padded per head: head hd lives in rows (hd%2)*64.. of
        # [:, hd, :]; the other 64 rows stay zero so S matmuls contract 128.
        QT2 = persist.tile([128, NH, TH], BF)
        nc.gpsimd.memset(QT2[:], 0.0)
        KT = persist.tile([128, DCH, T], BF)     # K^T rows=D, cols=all tok
        # V rows=tok, flat free layout: head hd / key-block kc block starts at
        # hd*520 + kc*65 = [V_h (64) | ones (1)]; the 128-wide PV stationary
        # slice reads 63 junk cols of the next block (junk output rows 65:128
        # are never read).  Padded so per-kc land DMAs stay in bounds.
        VA2 = persist.tile([128, 8776], BF)
        for hd in range(NH):
            nc.gpsimd.memset(
                VA2[:, hd * 520 + 64:hd * 520 + 64 + 8 * 65].rearrange(
                    "p (kc o) -> p kc o", o=65)[:, :, 0:1], 1.0)
        yT = persist.tile([128, DCH, TH], BF)    # attn out^T, rows=D

        # ---------------- helpers ----------------
        def layernorm(z_out, src):
            """z_out (sbuf bf16 [128, DCH, TH]) = LayerNorm(src) in D-major."""
            s_b = pp_mm.tile([128, TH], F32, tag="mm")
            for d in range(DCH):
                nc.tensor.matmul(s_b[:], ones128[:], src[:, d, :],
                                 start=(d == 0), stop=(d == DCH - 1))
            q_b = pp_mm.tile([128, TH], F32, tag="mm")
            for d in range(DCH):
                sq = tmp.tile([128, TH], F32R, tag="sq", bufs=1)
                nc.scalar.square(sq[:], src[:, d, :])
                nc.tensor.matmul(q_b[:], ones128[:], sq[:],
                                 start=(d == 0), stop=(d == DCH - 1))
            # ss = s^2 (scalar engine: only one PSUM port on DVE);
            # u = q - ss/D; std = sqrt(u/D + eps); rinv = 1/std; mr = (s/D)*rinv
            ss = stat.tile([128, TH], F32, tag="stat")
            nc.scalar.square(ss[:], s_b[:])
            u = stat.tile([128, TH], F32, tag="stat")
            nc.vector.scalar_tensor_tensor(u[:], ss[:], -1.0 / D, q_b[:],
                                           AluOp.mult, AluOp.add)
            rinv = stat.tile([128, TH], F32, tag="stat")
            nc.scalar.activation(rinv[:], u[:], Act.Sqrt, scale=1.0 / D,
                                 bias=eps_t[:])
            nc.vector.reciprocal_approx_fast(rinv[:], rinv[:])
            mr = u
            nc.vector.scalar_tensor_tensor(mr[:], s_b[:], 1.0 / D, rinv[:],
                                           AluOp.mult, AluOp.mult)
            for d in range(DCH):
                t = tmp.tile([128, TH], F32, tag="zt", bufs=2)
                nc.vector.tensor_tensor(t[:], src[:, d, :], rinv[:],
                                        AluOp.mult)
                nc.vector.tensor_tensor(z_out[:, d, :], t[:], mr[:],
                                        AluOp.subtract)

        # ---------------- embedding ----------------
        we3 = p_we.ap().rearrange("(c p) n -> p c n", p=128)      # [128, 4, 512]
        for r in range(4):
            wet = tmp.tile([128, 4, 128], BF, tag="tbf", bufs=2)
            nc.sync.dma_start(wet[:], we3[:, :, r * 128:(r + 1) * 128])
            ep = pp_mm.tile([128, TH], F32, tag="mm")
            for ec in range(4):
                lt = tmp.tile([128, TH], BF, tag="tbf", bufs=2)
                nc.sync.dma_start(lt[:], p_lcdT.ap()[ec * 128:(ec + 1) * 128, :])
                nc.tensor.matmul(ep[:], wet[:, ec, :], lt[:],
                                 start=(ec == 0), stop=(ec == 3))
            pt = tmp.tile([128, TH], F32, tag="t32", bufs=2)
            nc.sync.dma_start(pt[:], p_posT.ap()[r * 128:(r + 1) * 128, :])
            nc.vector.tensor_tensor(h[:, r, :], ep[:], pt[:], AluOp.add)
        actT = tmp.tile([AD, TH], F32, tag="t32", bufs=2)
        nc.sync.dma_start(actT[:], p_actT.ap())
        for r in range(4):
            wat = tmp.tile([AD, 128], F32, tag="t32", bufs=2)
            nc.sync.dma_start(wat[:], p_wa.ap()[:, r * 128:(r + 1) * 128])
            ap_ = pp_mm.tile([128, TH], F32, tag="mm")
            nc.tensor.matmul(ap_[:], wat[:], actT[:], start=True, stop=True)
            pt = tmp.tile([128, TH], F32, tag="t32", bufs=2)
            nc.sync.dma_start(pt[:], p_posT.ap()[(4 + r) * 128:(5 + r) * 128, :])
            nc.vector.tensor_tensor(h[:, 4 + r, :], ap_[:], pt[:], AluOp.add)

        # ---------------- transformer layers ----------------
        for l in range(NL):
            z1 = zpool.tile([128, DCH, TH], BF, tag="z", bufs=2)
            layernorm(z1, h)

            # ---- K^T own tokens, then AllGather over the pair ----
            wk3 = p_wk.ap()[l].rearrange("(c p) n -> p c n", p=128)
            kloc = stage.tile([128, DCH, TH], BF, tag="st8")
            for r in range(DCH):
                wt = wpool.tile([128, DCH, 128], BF, tag="w")
                nc.sync.dma_start(wt[:], wk3[:, :, r * 128:(r + 1) * 128])
                kp = pp_mm.tile([128, TH], F32, tag="mm")
                for d in range(DCH):
                    nc.tensor.matmul(kp[:], wt[:, d, :], z1[:, d, :],
                                     start=(d == 0), stop=(d == DCH - 1))
                nc.vector.tensor_copy(kloc[:, r, :], kp[:])
            kin = dram.tile([D, TH], BF, tag="kin")
            nc.sync.dma_start(kin.rearrange("(r p) t -> p r t", p=128), kloc[:])
            kout = dram.tile([2 * D, TH], BF, tag="kout")
            nc.gpsimd.collective_compute(
                "AllGather", AluOp.bypass,
                replica_groups=[[0, 1], [2, 3], [4, 5], [6, 7]],
                ins=[kin.opt()], outs=[kout.opt()])

            # ---- V own tokens (token-major), then AllGather ----
            wv3 = p_wv.ap()[l].rearrange("(c p) n -> p c n", p=128)
            vloc = stage.tile([128, TCH, D], BF, tag="st8")
            for nn in range(2):
                wvt = wvpool.tile([128, DCH, 512], BF, tag="wv8")
                nc.sync.dma_start(wvt[:], wv3[:, :, nn * 512:(nn + 1) * 512])
                for tb in range(TCH):
                    vp = pp_mm.tile([128, 512], F32, tag="mm")
                    for d in range(DCH):
                        nc.tensor.matmul(
                            vp[:], z1[:, d, tb * 128:(tb + 1) * 128],
                            wvt[:, d, :],
                            start=(d == 0), stop=(d == DCH - 1))
                    nc.vector.tensor_copy(vloc[:, tb, nn * 512:(nn + 1) * 512],
                                          vp[:])
            vin = dram.tile([TH, D], BF, tag="vin")
            nc.sync.dma_start(vin.rearrange("(c p) d -> p c d", p=128), vloc[:])
            vout = dram.tile([T, D], BF, tag="vout")
            nc.gpsimd.collective_compute(
                "AllGather", AluOp.bypass,
                replica_groups=[[0, 1], [2, 3], [4, 5], [6, 7]],
                ins=[vin.opt()], outs=[vout.opt()])

            # ---- Q^T own tokens into zero-padded per-head slots ----
            wq3 = p_wq.ap()[l].rearrange("(c p) n -> p c n", p=128)
            for r in range(DCH):
                wt = wpool.tile([128, DCH, 128], BF, tag="w")
                nc.sync.dma_start(wt[:], wq3[:, :, r * 128:(r + 1) * 128])
                qp = pp_mm.tile([128, TH], F32, tag="mm")
                for d in range(DCH):
                    nc.tensor.matmul(qp[:], wt[:, d, :], z1[:, d, :],
                                     start=(d == 0), stop=(d == DCH - 1))
                nc.vector.tensor_copy(QT2[0:64, 2 * r, :], qp[0:64, :])
                nc.vector.tensor_copy(QT2[64:128, 2 * r + 1, :], qp[64:128, :])

            # ---- land gathered K into KT (absolute token order) ----
            ko3 = kout.rearrange("(hh r p) t -> hh p r t", p=128, hh=2)
            for hh in range(2):
                nc.sync.dma_start(KT[:, :, hh * TH:(hh + 1) * TH], ko3[hh])
            # ---- land gathered V into VA2 (skip the ones columns) ----
            vo3 = vout.rearrange("(c p) (hd dh) -> c p hd dh", p=128, dh=DH)
            for c in range(8):
                nc.sync.dma_start(
                    VA2[:, c * 65:c * 65 + NH * 520].rearrange(
                        "p (hd x) -> p hd x", x=520)[:, :, 0:64],
                    vo3[c])

            # ---- attention ----
            for hd in range(NH):
                rc, ro = hd // 2, (hd % 2) * 64
                o_p = pp_o.tile([128, TH], F32, tag="o")
                p2s = []
                for kcp in range(4):
                    s2 = pp_pair.tile([128, 2, TH], F32, tag="pair")
                    for j in range(2):
                        kc = kcp * 2 + j
                        nc.tensor.matmul(
                            s2[:, j, :],
                            KT[:, rc, kc * 128:(kc + 1) * 128],
                            QT2[:, hd, :],
                            start=True, stop=True)
                    p2 = ptp.tile([128, 2, TH], BF, tag="p2", bufs=4)
                    nc.scalar.activation(p2[:], s2[:], Act.Exp,
                                         scale=1.0 / float(np.sqrt(DH)))
                    nc.vector.tensor_tensor(p2[:], p2[:],
                                            mask01[:, 2 * kcp:2 * kcp + 2, :],
                                            AluOp.mult)
                    p2s.append(p2)
                for kcp in range(4):
                    for j in range(2):
                        kc = kcp * 2 + j
                        nc.tensor.matmul(
                            o_p[:],
                            VA2[:, hd * 520 + kc * 65:hd * 520 + kc * 65 + 128],
                            p2s[kcp][:, j, :],
                            start=(kcp == 0 and j == 0),
                            stop=(kcp == 3 and j == 1))
                # normalize: row 64 of o_p is the denominator; invert it,
                # replicate to 64 partitions with a 0-stride DMA, then one
                # multiply writes yT.
                inv1 = stat.tile([1, TH], F32, tag="den1", bufs=2)
                nc.vector.tensor_copy(inv1[:], o_p[64:65, :])
                nc.vector.reciprocal_approx_fast(inv1[:], inv1[:])
                invO = stat.tile([64, TH], F32, tag="den", bufs=2)
                nc.gpsimd.partition_broadcast(invO[:], inv1[0:1, :])
                nc.vector.tensor_tensor(yT[ro:ro + 64, rc, :], o_p[0:64, :],
                                        invO[:], AluOp.mult)

            # ---- proj + residual ----
            wp3 = p_wp.ap()[l].rearrange("(c p) n -> p c n", p=128)
            for r in range(DCH):
                wt = wpool.tile([128, DCH, 128], BF, tag="w")
                nc.sync.dma_start(wt[:], wp3[:, :, r * 128:(r + 1) * 128])
                pp = pp_mm.tile([128, TH], F32, tag="mm")
                for d in range(DCH):
                    nc.tensor.matmul(pp[:], wt[:, d, :], yT[:, d, :],
                                     start=(d == 0), stop=(d == DCH - 1))
                nc.vector.tensor_tensor(h[:, r, :], h[:, r, :], pp[:],
                                        AluOp.add)

            # ---- MLP ----
            z2 = zpool.tile([128, DCH, TH], BF, tag="z", bufs=2)
            layernorm(z2, h)
            aT = big.tile([128, 32, TH], BF, tag="aT")
            w13 = p_w1.ap()[l].rearrange("(c p) f -> p c f", p=128)
            for fi in range(16):
                w1t = wpool.tile([128, DCH, 256], BF, tag="w1p", bufs=2)
                nc.sync.dma_start(w1t[:], w13[:, :, fi * 256:(fi + 1) * 256])
                fp = pp_pair.tile([128, 2, TH], F32, tag="pair")
                for j in range(2):
                    for d in range(DCH):
                        nc.tensor.matmul(
                            fp[:, j, :], w1t[:, d, j * 128:(j + 1) * 128],
                            z2[:, d, :],
                            start=(d == 0), stop=(d == DCH - 1))
                nc.scalar.activation(aT[:, 2 * fi:2 * fi + 2, :], fp[:],
                                     Act.Gelu)
            w23 = p_w2.ap()[l].rearrange("(c p) n -> p c n", p=128)
            for r in range(DCH):
                w2t = wvpool.tile([128, FF // 128, 128], BF, tag="wv8")
                nc.sync.dma_start(w2t[:], w23[:, :, r * 128:(r + 1) * 128])
                mp = pp_mm.tile([128, TH], F32, tag="mm")
                for fc in range(32):
                    nc.tensor.matmul(mp[:], w2t[:, fc, :], aT[:, fc, :],
                                     start=(fc == 0), stop=(fc == 31))
                nc.vector.tensor_tensor(h[:, r, :], h[:, r, :], mp[:],
                                        AluOp.add)

        # ---------------- final LN + head ----------------
        zf = zpool.tile([128, DCH, TH], BF, tag="z", bufs=2)
        layernorm(zf, h)
        wht = wvpool.tile([128, DCH, E], BF, tag="wv8")
        nc.sync.dma_start(wht[:], p_wh.ap().rearrange("(c p) e -> p c e", p=128))
        for tb in range(TCH):
            op_ = pp_mm.tile([128, E], F32, tag="mm")
            for d in range(DCH):
                nc.tensor.matmul(
                    op_[:],
                    zf[:, d, tb * 128:(tb + 1) * 128],
                    wht[:, d, :],
                    start=(d == 0), stop=(d == DCH - 1))
            ot = tmp.tile([128, E], F32, tag="t32", bufs=2)
            nc.scalar.copy(ot[:], op_[:])
            nc.sync.dma_start(p_out.ap()[tb * 128:(tb + 1) * 128, :], ot[:])

        for _pool in reversed((const, persist, zpool, big, wpool, wvpool,
                               stage, tmp, stat, ptp, dram, pp_mm, pp_pair,
                               pp_o)):
            _pool.release()

    nc.compile()
    return nc


def _get_program():
    if "nc" not in _cache:
        _cache["nc"] = _build_program()
    return _cache["nc"]


def _bf16(x):
    return np.ascontiguousarray(np.asarray(x).astype(ml_dtypes.bfloat16))


def _f32(x):
    return np.ascontiguousarray(np.asarray(x).astype(np.float32))


def make_in_maps(inputs):
    lcd = np.asarray(inputs["lcd"], np.float32).reshape(B, T, E)
    lcd_shift = np.concatenate(
        [np.zeros((B, 1, E), np.float32), lcd[:, :-1]], axis=1)
    action = np.asarray(inputs["action"], np.float32)
    pos = np.asarray(inputs["pos_emb"], np.float32)[0]          # [T, D]

    shared = {
        "W_embed": _bf16(inputs["W_embed"]),
        "W_act": _f32(inputs["W_act"]),
        "Wq": _bf16(inputs["Wq"]),
        "Wk": _bf16(inputs["Wk"]),
        "Wv": _bf16(inputs["Wv"]),
        "Wp": _bf16(inputs["Wp"]),
        "W1": _bf16(inputs["W1"]),
        "W2": _bf16(inputs["W2"]),
        "Wh": _bf16(inputs["Wh"]),
        "ones128": np.ones((128, 128), np.float32),
    }

    in_maps = []
    for c in range(NC):
        b, half = c // 2, c % 2
        tok = np.arange(half * TH, (half + 1) * TH)             # abs own tokens
        kabs = np.arange(T)                                     # abs key index
        # multiplicative 0/1 causal mask in S^T layout: [k-block, 128 k, TH q]
        m = (kabs[:, None] <= tok[None, :]).astype(np.float32)
        mask01 = m.reshape(8, 128, TH)
        in_maps.append(dict(
            shared,
            lcdT=_bf16(lcd_shift[b, tok].T),                    # [E, TH]
            actT=_f32(action[b, tok].T),                        # [AD, TH]
            posT=_f32(pos[tok].T),                              # [D, TH]
            mask01=_bf16(mask01),
        ))
    return in_maps


def assemble(results):
    out = np.empty((B, T, E), np.float32)
    for c in range(NC):
        b, half = c // 2, c % 2
        out[b, half * TH:(half + 1) * TH] = results[c]["out"]
    return out


def kernel(**inputs):
    nc = _get_program()
    in_maps = make_in_maps(inputs)
    res = run_bass_kernel_spmd(nc, in_maps, list(range(NC)))
    return assemble(res.results)
